# revision 12
# baseline (speedup 1.0000x reference)
"""HGNN layer on 8 Trainium2 NeuronCores (Bass/Tile) — v3.

Reference computation:
    x1 = x @ W1                                    [N, F]
    w = softmax(where(seq > 0, 1, -9e15))          uniform over valid slots
    edge = relu(sum_l w[e,l] * x1[seq[e,l]])       [E, F]
    e1 = edge @ W2                                 [E, F]
    uw = softmax(where(useq > 0, 1, -9e15))
    node = sum_l uw[n,l] * e1[useq[n,l]]           [N, F]

v3 design (vs the v1 baseline at 4.13ms):
  - All gathers use the ucode-accelerated InstDMAGatherAnt (`dma_gather`):
    one instruction per 128-row tile (4096 int16 indices) instead of 32
    separate indirect DMAs. SWDGE emission cost: 994ns + 0.34ns/desc.
  - By linearity, edge = (scale * sum_l x[seq[e,l]]) @ W1 — so stage 1
    gathers RAW x rows and applies W1 after the reduce. No x1 table and
    no stage-0 matmul/AllGather at all.
  - dma_gather indices are int16 (max 32767). x has 50000 rows, so it is
    split into lo/hi half-tables; each tile issues two gathers with
    dummy zero-row indices for out-of-half slots, merged by one add.
    The e1 table (25600 rows) fits int16 directly.
  - Softmax weights / empty rows / padding fold into host-precomputed
    index remaps + per-row scales; device epilogue is one fused
    tensor_scalar (scale [+relu]) per tile.
  - fp16 on device (tables, matmuls, tree reduce); PSUM accumulation in
    fp32. rel-err budget is 2e-2; measured ~2e-3.
"""

import sys

sys.path.insert(0, "/opt/trn_rl_repo")

import numpy as np

N = 50000
E = 25000
F = 256
L = 32
P = 128
NC_COUNT = 8
NSH = N // NC_COUNT        # 6250 nodes per core
ESH = E // NC_COUNT        # 3125 edges per core
NT = 49                    # stage2 tiles per core (6272 rows)
ET = 25                    # stage1 tiles per core (3200 rows)
NSH_PAD = NT * P           # 6272
ESH_PAD = ET * P           # 3200
NX = 50048                 # x rows padded (zero rows 50000..50047)
XLO = 25024                # rows 0..25023 in xlo; rest in xhi
NLO = XLO + 1              # xlo rows (+1 appended zero row)
NHI = NX - XLO             # 25024 rows; trailing rows are zero padding
DUM_LO = NLO - 1           # appended zero row
DUM_HI = NHI - 1           # node 50047 -> zero
NE1 = NC_COUNT * ESH_PAD   # 25600 e1 table rows
E1_ZERO = ESH_PAD - 1      # local row (p=127, t=24): edge 3199 >= ESH -> zeros
NIDX = P * L               # 4096 indices per gather tile
IW = NIDX // 16            # idx columns per tile in wrapped layout (256)


def build_program():
    from concourse import bacc, bass, mybir, tile  # noqa: F401
    from concourse.masks import make_identity

    fp32 = mybir.dt.float32
    fp16 = mybir.dt.float16
    i16 = mybir.dt.int16

    nc = bacc.Bacc("TRN2", target_bir_lowering=False, debug=False,
                   num_devices=NC_COUNT, num_swdge_queues=4)

    xlo = nc.dram_tensor("xlo", [NLO, F], fp16, kind="ExternalInput").ap()
    xhi = nc.dram_tensor("xhi", [NHI, F], fp16, kind="ExternalInput").ap()
    w1 = nc.dram_tensor("w1", [F, F], fp16, kind="ExternalInput").ap()
    w2 = nc.dram_tensor("w2", [F, F], fp16, kind="ExternalInput").ap()
    silo = nc.dram_tensor("silo", [P, ET * IW], i16,
                          kind="ExternalInput").ap()
    sihi = nc.dram_tensor("sihi", [P, ET * IW], i16,
                          kind="ExternalInput").ap()
    seqs = nc.dram_tensor("seqs", [P, ET], fp32, kind="ExternalInput").ap()
    useqi = nc.dram_tensor("useqi", [P, NT * IW], i16,
                           kind="ExternalInput").ap()
    useqs = nc.dram_tensor("useqs", [P, NT], fp32, kind="ExternalInput").ap()
    out = nc.dram_tensor("out", [P, NT, F], fp16, kind="ExternalOutput").ap()

    AL = mybir.AluOpType

    with tile.TileContext(nc) as tc:
        with (
            tc.tile_pool(name="cst", bufs=1) as cst,
            tc.tile_pool(name="gb", bufs=2) as gbp,
            tc.tile_pool(name="gb2", bufs=3) as gbp2,
            tc.tile_pool(name="sb", bufs=3) as sbp,
            tc.tile_pool(name="acc", bufs=1) as accp,
            tc.tile_pool(name="ps", bufs=3, space="PSUM") as psp,
            tc.tile_pool(name="pss", bufs=2, space="PSUM") as pssp,
            tc.tile_pool(name="pst", bufs=2, space="PSUM") as pstp,
            tc.tile_pool(name="dram", bufs=1, space="DRAM") as dram,
        ):
            # ---------- constants ----------
            ident = cst.tile([P, P], fp16)
            make_identity(nc, ident[:])
            w1sb = [cst.tile([P, F], fp16, name=f"w1k{k}") for k in range(2)]
            w2sb = [cst.tile([P, F], fp16, name=f"w2k{k}") for k in range(2)]
            for k in range(2):
                nc.sync.dma_start(out=w1sb[k][:], in_=w1[k * P:(k + 1) * P, :])
                nc.sync.dma_start(out=w2sb[k][:], in_=w2[k * P:(k + 1) * P, :])
            silo_sb = cst.tile([P, ET * IW], i16)
            nc.sync.dma_start(out=silo_sb[:], in_=silo[:, :])
            sihi_sb = cst.tile([P, ET * IW], i16)
            nc.sync.dma_start(out=sihi_sb[:], in_=sihi[:, :])
            seqs_sb = cst.tile([P, ET], fp32)
            nc.sync.dma_start(out=seqs_sb[:], in_=seqs[:, :])
            useqi_sb = cst.tile([P, NT * IW], i16)
            nc.sync.dma_start(out=useqi_sb[:], in_=useqi[:, :])
            useqs_sb = cst.tile([P, NT], fp32)
            nc.sync.dma_start(out=useqs_sb[:], in_=useqs[:, :])

            # ---------- DRAM scratch ----------
            e1loc = dram.tile([ESH_PAD, F], fp16)
            e1tab = dram.tile([NE1, F], fp16, addr_space="Shared")


            def trans_mm(src, wsb, ps):
                """ps += src @ W  (PE transpose then matmul, 2 k-chunks)."""
                for k in range(2):
                    pst = pstp.tile([P, P], fp16, tag="tr")
                    nc.tensor.transpose(
                        out=pst[:], in_=src[:, k * P:(k + 1) * P],
                        identity=ident[:])
                    srcT = sbp.tile([P, P], fp16, tag="srcT")
                    nc.vector.tensor_copy(out=srcT[:], in_=pst[:])
                    nc.tensor.matmul(ps[:], srcT[:], wsb[k][:],
                                     start=(k == 0), stop=(k == 1))

            # ---------- stage 1: edges ----------
            with nc.named_scope("stage1"):
                e1sl = accp.tile([P, ET, F], fp16, name="e1sl")
                for t in range(ET):
                    g = gbp.tile([P, 2 * L, F], fp16, tag="gb")
                    nc.gpsimd.dma_gather(
                        g[:, 0:L, :], xlo[:, :],
                        silo_sb[:, t * IW:(t + 1) * IW],
                        NIDX, NIDX, F, single_packet=False,
                        queue_num=(2 * t) % 4)
                    nc.gpsimd.dma_gather(
                        g[:, L:2 * L, :], xhi[:, :],
                        sihi_sb[:, t * IW:(t + 1) * IW],
                        NIDX, NIDX, F, single_packet=False,
                        queue_num=(2 * t + 1) % 4)
                    # slot-sum on PE: psum += I @ g[:, k, :] for all 64 slots
                    pss = pssp.tile([P, F], fp32, tag="sum")
                    for k in range(2 * L):
                        nc.tensor.matmul(pss[:], ident[:], g[:, k, :],
                                         start=(k == 0), stop=(k == 2 * L - 1))
                    sum1 = sbp.tile([P, F], fp16, tag="sum1")
                    nc.vector.tensor_copy(out=sum1[:], in_=pss[:])
                    # edge = relu((sum @ W1) * (1/cnt)); scale>0 commutes
                    ps1 = psp.tile([P, F], fp32, tag="mm")
                    trans_mm(sum1[:], w1sb, ps1)
                    edge = sbp.tile([P, F], fp16, tag="edge")
                    nc.vector.tensor_scalar(
                        out=edge[:], in0=ps1[:],
                        scalar1=seqs_sb[:, t:t + 1], scalar2=0.0,
                        op0=AL.mult, op1=AL.max)
                    # e1 = edge @ W2
                    ps2 = psp.tile([P, F], fp32, tag="mm")
                    trans_mm(edge[:], w2sb, ps2)
                    nc.vector.tensor_copy(out=e1sl[:, t, :], in_=ps2[:])
                v = e1loc[:, :].rearrange("(t p) f -> p t f", p=P)
                nc.sync.dma_start(out=v[:, :, :], in_=e1sl[:])
                nc.gpsimd.collective_compute(
                    "AllGather", AL.bypass,
                    replica_groups=[list(range(NC_COUNT))],
                    ins=[e1loc.opt()], outs=[e1tab.opt()],
                )

            # ---------- stage 2: nodes ----------
            with nc.named_scope("stage2"):
                for t in range(NT):
                    g = gbp2.tile([P, L, F], fp16, tag="gb2")
                    nc.gpsimd.dma_gather(
                        g[:, :, :], e1tab[:, :],
                        useqi_sb[:, t * IW:(t + 1) * IW],
                        NIDX, NIDX, F, single_packet=False,
                        queue_num=t % 4)
                    pss = pssp.tile([P, F], fp32, tag="sum")
                    for k in range(L):
                        nc.tensor.matmul(pss[:], ident[:], g[:, k, :],
                                         start=(k == 0), stop=(k == L - 1))
                    osb = sbp.tile([P, F], fp16, tag="ot")
                    nc.scalar.activation(
                        out=osb[:], in_=pss[:],
                        func=mybir.ActivationFunctionType.Copy,
                        scale=useqs_sb[:, t:t + 1])
                    nc.sync.dma_start(out=out[:, t, :], in_=osb[:])

    nc.compile()
    return nc


def _remap_e1(gid):
    """edge id -> e1tab row (t-major): r = c*3200 + local edge id."""
    return (gid // ESH) * ESH_PAD + gid % ESH


def _slots_scale(idx_rows, remap, zero_row, n_rows_pad):
    """Remap [rows, L] ids to table rows + per-row 1/cnt scale.

    Padding slots (id 0) -> zero_row (or -1 sentinel if zero_row is None);
    all-padding rows -> table row of id 0 with scale 1/L (matches the
    reference softmax exactly). Pad rows get scale 0."""
    rows = idx_rows.shape[0]
    valid = idx_rows > 0
    cnt = valid.sum(axis=1)
    fill = -1 if zero_row is None else zero_row
    slots = np.where(valid, remap(idx_rows), fill).astype(np.int64)
    empty = cnt == 0
    if empty.any():
        slots[empty] = remap(np.zeros((1, 1), dtype=idx_rows.dtype))
    scale = 1.0 / np.maximum(cnt, 1)
    scale[empty] = 1.0 / L

    slots_pad = np.full((n_rows_pad, L), fill, np.int64)
    slots_pad[:rows] = np.sort(slots, axis=1)
    scale_pad = np.zeros((n_rows_pad,), np.float32)
    scale_pad[:rows] = scale
    nt = n_rows_pad // P
    sc_sb = np.ascontiguousarray(scale_pad.reshape(nt, P).T)
    return slots_pad, sc_sb


def _wrap_streams(slots_pad):
    """[nt*128, L] table rows -> int16 idx tiles in dma_gather layout.

    Per tile: stream[i] = slots[tile*128 + i%128, i//128]; element i sits
    at [i%16, i//16] of a [16, NIDX/16] block, tiled to 128 partitions."""
    n_rows, l = slots_pad.shape
    nt = n_rows // P
    # stream per tile: s[t, i] = slots[t*128 + i%128, i//128]
    s = slots_pad.reshape(nt, P, l).transpose(0, 2, 1).reshape(nt, P * l)
    # wrap: element i -> [i%16, i//16]
    blk = s.reshape(nt, P * l // 16, 16).transpose(0, 2, 1)  # [nt, 16, IW]
    wrapped = np.concatenate(list(blk), axis=1)              # [16, nt*IW]
    return np.ascontiguousarray(np.tile(wrapped, (8, 1)).astype(np.int16))


def make_in_maps(x, seq, useq, W1, W2):
    x = np.asarray(x, dtype=np.float32)
    W1 = np.asarray(W1, dtype=np.float32)
    W2 = np.asarray(W2, dtype=np.float32)
    seq = np.asarray(seq)
    useq = np.asarray(useq)

    xp = np.zeros((NX, F), np.float16)
    xp[:N] = x
    xlo = np.zeros((NLO, F), np.float16)
    xlo[:XLO] = xp[:XLO]
    xhi = np.ascontiguousarray(xp[XLO:])
    w1_16 = W1.astype(np.float16)
    w2_16 = W2.astype(np.float16)

    in_maps = []
    for c in range(NC_COUNT):
        slots, ss = _slots_scale(seq[c * ESH:(c + 1) * ESH],
                                 lambda g: g, None, ESH_PAD)
        lo = np.where((slots >= 0) & (slots < XLO), slots, DUM_LO)
        hi = np.where(slots >= XLO, slots - XLO, DUM_HI)
        uslots, us = _slots_scale(useq[c * NSH:(c + 1) * NSH], _remap_e1,
                                  E1_ZERO, NSH_PAD)
        in_maps.append({
            "xlo": xlo,
            "xhi": xhi,
            "w1": w1_16,
            "w2": w2_16,
            "silo": _wrap_streams(lo),
            "sihi": _wrap_streams(hi),
            "seqs": ss,
            "useqi": _wrap_streams(uslots),
            "useqs": us,
        })
    return in_maps


def assemble(results):
    """[P, NT, F] p-major slabs per core -> [N, F] full output."""
    parts = []
    for c in range(NC_COUNT):
        arr = np.asarray(results[c]["out"]).astype(np.float32)
        parts.append(arr.transpose(1, 0, 2).reshape(NSH_PAD, F)[:NSH])
    return np.concatenate(parts, axis=0)


def kernel(x, seq, useq, W1, W2):
    from concourse.bass_utils import run_bass_kernel_spmd

    in_maps = make_in_maps(x, seq, useq, W1, W2)
    nc = build_program()
    res = run_bass_kernel_spmd(nc, in_maps, core_ids=list(range(NC_COUNT)),
                               trace=False)
    return assemble(res.results)


# revision 13
# speedup vs baseline: 1.3640x; 1.3640x over previous
"""HGNN layer on 8 Trainium2 NeuronCores (Bass/Tile) — v3.

Reference computation:
    x1 = x @ W1                                    [N, F]
    w = softmax(where(seq > 0, 1, -9e15))          uniform over valid slots
    edge = relu(sum_l w[e,l] * x1[seq[e,l]])       [E, F]
    e1 = edge @ W2                                 [E, F]
    uw = softmax(where(useq > 0, 1, -9e15))
    node = sum_l uw[n,l] * e1[useq[n,l]]           [N, F]

v3 design (vs the v1 baseline at 4.13ms):
  - All gathers use the ucode-accelerated InstDMAGatherAnt (`dma_gather`):
    one instruction per 128-row tile (4096 int16 indices) instead of 32
    separate indirect DMAs. SWDGE emission cost: 994ns + 0.34ns/desc.
  - By linearity, edge = (scale * sum_l x[seq[e,l]]) @ W1 — so stage 1
    gathers RAW x rows and applies W1 after the reduce. No x1 table and
    no stage-0 matmul/AllGather at all.
  - dma_gather indices are int16 (max 32767). x has 50000 rows, so it is
    split into lo/hi half-tables; each tile issues two gathers with
    dummy zero-row indices for out-of-half slots, merged by one add.
    The e1 table (25600 rows) fits int16 directly.
  - Softmax weights / empty rows / padding fold into host-precomputed
    index remaps + per-row scales; device epilogue is one fused
    tensor_scalar (scale [+relu]) per tile.
  - fp16 on device (tables, matmuls, tree reduce); PSUM accumulation in
    fp32. rel-err budget is 2e-2; measured ~2e-3.
"""

import sys

sys.path.insert(0, "/opt/trn_rl_repo")

import numpy as np

N = 50000
E = 25000
F = 256
L = 32
P = 128
NC_COUNT = 8
NSH = N // NC_COUNT        # 6250 nodes per core
ESH = E // NC_COUNT        # 3125 edges per core
NT = 49                    # stage2 tiles per core (6272 rows)
ET = 25                    # stage1 tiles per core (3200 rows)
NSH_PAD = NT * P           # 6272
ESH_PAD = ET * P           # 3200
NX = 50048                 # x rows padded (zero rows 50000..50047)
XLO = 25024                # rows 0..25023 in xlo; rest in xhi
NLO = XLO + 1              # xlo rows (+1 appended zero row)
NHI = NX - XLO             # 25024 rows; trailing rows are zero padding
DUM_LO = NLO - 1           # appended zero row
DUM_HI = NHI - 1           # node 50047 -> zero
NE1 = NC_COUNT * ESH_PAD   # 25600 e1 table rows
E1_ZERO = ESH_PAD - 1      # local row (p=127, t=24): edge 3199 >= ESH -> zeros
NIDX = P * L               # 4096 indices per gather tile
IW = NIDX // 16            # idx columns per tile in wrapped layout (256)


def build_program():
    from concourse import bacc, bass, mybir, tile  # noqa: F401
    from concourse.masks import make_identity

    fp32 = mybir.dt.float32
    fp16 = mybir.dt.float16
    i16 = mybir.dt.int16

    nc = bacc.Bacc("TRN2", target_bir_lowering=False, debug=False,
                   num_devices=NC_COUNT, num_swdge_queues=4)

    xlo = nc.dram_tensor("xlo", [NLO, F], fp16, kind="ExternalInput").ap()
    xhi = nc.dram_tensor("xhi", [NHI, F], fp16, kind="ExternalInput").ap()
    w1 = nc.dram_tensor("w1", [F, F], fp16, kind="ExternalInput").ap()
    w2 = nc.dram_tensor("w2", [F, F], fp16, kind="ExternalInput").ap()
    silo = nc.dram_tensor("silo", [P, ET * IW], i16,
                          kind="ExternalInput").ap()
    sihi = nc.dram_tensor("sihi", [P, ET * IW], i16,
                          kind="ExternalInput").ap()
    seqs = nc.dram_tensor("seqs", [P, ET], fp32, kind="ExternalInput").ap()
    useqi = nc.dram_tensor("useqi", [P, NT * IW], i16,
                           kind="ExternalInput").ap()
    useqs = nc.dram_tensor("useqs", [P, NT], fp32, kind="ExternalInput").ap()
    out = nc.dram_tensor("out", [P, NT, F], fp16, kind="ExternalOutput").ap()

    AL = mybir.AluOpType

    with tile.TileContext(nc) as tc:
        with (
            tc.tile_pool(name="cst", bufs=1) as cst,
            tc.tile_pool(name="gb", bufs=2) as gbp,
            tc.tile_pool(name="gb2", bufs=3) as gbp2,
            tc.tile_pool(name="sb", bufs=3) as sbp,
            tc.tile_pool(name="acc", bufs=1) as accp,
            tc.tile_pool(name="ps", bufs=4, space="PSUM") as psp,
            tc.tile_pool(name="pst", bufs=2, space="PSUM") as pstp,
            tc.tile_pool(name="dram", bufs=1, space="DRAM") as dram,
        ):
            # ---------- constants ----------
            ident = cst.tile([P, P], fp16)
            make_identity(nc, ident[:])
            w1sb = [cst.tile([P, F], fp16, name=f"w1k{k}") for k in range(2)]
            w2sb = [cst.tile([P, F], fp16, name=f"w2k{k}") for k in range(2)]
            for k in range(2):
                nc.sync.dma_start(out=w1sb[k][:], in_=w1[k * P:(k + 1) * P, :])
                nc.sync.dma_start(out=w2sb[k][:], in_=w2[k * P:(k + 1) * P, :])
            silo_sb = cst.tile([P, ET * IW], i16)
            nc.sync.dma_start(out=silo_sb[:], in_=silo[:, :])
            sihi_sb = cst.tile([P, ET * IW], i16)
            nc.sync.dma_start(out=sihi_sb[:], in_=sihi[:, :])
            seqs_sb = cst.tile([P, ET], fp32)
            nc.sync.dma_start(out=seqs_sb[:], in_=seqs[:, :])
            useqi_sb = cst.tile([P, NT * IW], i16)
            nc.sync.dma_start(out=useqi_sb[:], in_=useqi[:, :])
            useqs_sb = cst.tile([P, NT], fp32)
            nc.sync.dma_start(out=useqs_sb[:], in_=useqs[:, :])

            # ---------- DRAM scratch ----------
            e1loc = dram.tile([ESH_PAD, F], fp16)
            e1tab = dram.tile([NE1, F], fp16, addr_space="Shared")


            def trans_mm(src, wsb, ps):
                """ps += src @ W  (PE transpose then matmul, 2 k-chunks)."""
                for k in range(2):
                    pst = pstp.tile([P, P], fp16, tag="tr")
                    nc.tensor.transpose(
                        out=pst[:], in_=src[:, k * P:(k + 1) * P],
                        identity=ident[:])
                    srcT = sbp.tile([P, P], fp16, tag="srcT")
                    nc.vector.tensor_copy(out=srcT[:], in_=pst[:])
                    nc.tensor.matmul(ps[:], srcT[:], wsb[k][:],
                                     start=(k == 0), stop=(k == 1))

            # ---------- stage 1: edges ----------
            with nc.named_scope("stage1"):
                e1sl = accp.tile([P, ET, F], fp16, name="e1sl")
                for t in range(ET):
                    g = gbp.tile([P, 2 * L, F], fp16, tag="gb")
                    nc.gpsimd.dma_gather(
                        g[:, 0:L, :], xlo[:, :],
                        silo_sb[:, t * IW:(t + 1) * IW],
                        NIDX, NIDX, F, single_packet=False,
                        queue_num=(2 * t) % 4)
                    nc.gpsimd.dma_gather(
                        g[:, L:2 * L, :], xhi[:, :],
                        sihi_sb[:, t * IW:(t + 1) * IW],
                        NIDX, NIDX, F, single_packet=False,
                        queue_num=(2 * t + 1) % 4)
                    h = 2 * L
                    while h > 2:
                        h //= 2
                        nc.vector.tensor_tensor(
                            out=g[:, 0:h, :], in0=g[:, 0:h, :],
                            in1=g[:, h:2 * h, :], op=AL.add)
                    sum1 = sbp.tile([P, F], fp16, tag="sum1")
                    nc.vector.tensor_tensor(
                        out=sum1[:], in0=g[:, 0, :], in1=g[:, 1, :], op=AL.add)
                    # edge = relu((sum @ W1) * (1/cnt)); scale>0 commutes
                    ps1 = psp.tile([P, F], fp32, tag="mm")
                    trans_mm(sum1[:], w1sb, ps1)
                    edge = sbp.tile([P, F], fp16, tag="edge")
                    nc.vector.tensor_scalar(
                        out=edge[:], in0=ps1[:],
                        scalar1=seqs_sb[:, t:t + 1], scalar2=0.0,
                        op0=AL.mult, op1=AL.max)
                    # e1 = edge @ W2
                    ps2 = psp.tile([P, F], fp32, tag="mm")
                    trans_mm(edge[:], w2sb, ps2)
                    nc.vector.tensor_copy(out=e1sl[:, t, :], in_=ps2[:])
                v = e1loc[:, :].rearrange("(t p) f -> p t f", p=P)
                nc.sync.dma_start(out=v[:, :, :], in_=e1sl[:])
                nc.gpsimd.collective_compute(
                    "AllGather", AL.bypass,
                    replica_groups=[list(range(NC_COUNT))],
                    ins=[e1loc.opt()], outs=[e1tab.opt()],
                )

            # ---------- stage 2: nodes ----------
            with nc.named_scope("stage2"):
                for t in range(NT):
                    g = gbp2.tile([P, L, F], fp16, tag="gb2")
                    nc.gpsimd.dma_gather(
                        g[:, :, :], e1tab[:, :],
                        useqi_sb[:, t * IW:(t + 1) * IW],
                        NIDX, NIDX, F, single_packet=False,
                        queue_num=t % 4)
                    h = L
                    while h > 2:
                        h //= 2
                        nc.vector.tensor_tensor(
                            out=g[:, 0:h, :], in0=g[:, 0:h, :],
                            in1=g[:, h:2 * h, :], op=AL.add)
                    sum2 = sbp.tile([P, F], fp16, tag="sum2")
                    nc.vector.tensor_tensor(
                        out=sum2[:], in0=g[:, 0, :], in1=g[:, 1, :], op=AL.add)
                    osb = sbp.tile([P, F], fp16, tag="ot")
                    nc.scalar.activation(
                        out=osb[:], in_=sum2[:],
                        func=mybir.ActivationFunctionType.Copy,
                        scale=useqs_sb[:, t:t + 1])
                    nc.sync.dma_start(out=out[:, t, :], in_=osb[:])

    nc.compile()
    return nc


def _remap_e1(gid):
    """edge id -> e1tab row (t-major): r = c*3200 + local edge id."""
    return (gid // ESH) * ESH_PAD + gid % ESH


def _slots_scale(idx_rows, remap, zero_row, n_rows_pad):
    """Remap [rows, L] ids to table rows + per-row 1/cnt scale.

    Padding slots (id 0) -> zero_row (or -1 sentinel if zero_row is None);
    all-padding rows -> table row of id 0 with scale 1/L (matches the
    reference softmax exactly). Pad rows get scale 0."""
    rows = idx_rows.shape[0]
    valid = idx_rows > 0
    cnt = valid.sum(axis=1)
    fill = -1 if zero_row is None else zero_row
    slots = np.where(valid, remap(idx_rows), fill).astype(np.int64)
    empty = cnt == 0
    if empty.any():
        slots[empty] = remap(np.zeros((1, 1), dtype=idx_rows.dtype))
    scale = 1.0 / np.maximum(cnt, 1)
    scale[empty] = 1.0 / L

    slots_pad = np.full((n_rows_pad, L), fill, np.int64)
    slots_pad[:rows] = np.sort(slots, axis=1)
    scale_pad = np.zeros((n_rows_pad,), np.float32)
    scale_pad[:rows] = scale
    nt = n_rows_pad // P
    sc_sb = np.ascontiguousarray(scale_pad.reshape(nt, P).T)
    return slots_pad, sc_sb


def _wrap_streams(slots_pad):
    """[nt*128, L] table rows -> int16 idx tiles in dma_gather layout.

    Per tile: stream[i] = slots[tile*128 + i%128, i//128]; element i sits
    at [i%16, i//16] of a [16, NIDX/16] block, tiled to 128 partitions."""
    n_rows, l = slots_pad.shape
    nt = n_rows // P
    # stream per tile: s[t, i] = slots[t*128 + i%128, i//128]
    s = slots_pad.reshape(nt, P, l).transpose(0, 2, 1).reshape(nt, P * l)
    # wrap: element i -> [i%16, i//16]
    blk = s.reshape(nt, P * l // 16, 16).transpose(0, 2, 1)  # [nt, 16, IW]
    wrapped = np.concatenate(list(blk), axis=1)              # [16, nt*IW]
    return np.ascontiguousarray(np.tile(wrapped, (8, 1)).astype(np.int16))


def make_in_maps(x, seq, useq, W1, W2):
    x = np.asarray(x, dtype=np.float32)
    W1 = np.asarray(W1, dtype=np.float32)
    W2 = np.asarray(W2, dtype=np.float32)
    seq = np.asarray(seq)
    useq = np.asarray(useq)

    xp = np.zeros((NX, F), np.float16)
    xp[:N] = x
    xlo = np.zeros((NLO, F), np.float16)
    xlo[:XLO] = xp[:XLO]
    xhi = np.ascontiguousarray(xp[XLO:])
    w1_16 = W1.astype(np.float16)
    w2_16 = W2.astype(np.float16)

    in_maps = []
    for c in range(NC_COUNT):
        slots, ss = _slots_scale(seq[c * ESH:(c + 1) * ESH],
                                 lambda g: g, None, ESH_PAD)
        lo = np.where((slots >= 0) & (slots < XLO), slots, DUM_LO)
        hi = np.where(slots >= XLO, slots - XLO, DUM_HI)
        uslots, us = _slots_scale(useq[c * NSH:(c + 1) * NSH], _remap_e1,
                                  E1_ZERO, NSH_PAD)
        in_maps.append({
            "xlo": xlo,
            "xhi": xhi,
            "w1": w1_16,
            "w2": w2_16,
            "silo": _wrap_streams(lo),
            "sihi": _wrap_streams(hi),
            "seqs": ss,
            "useqi": _wrap_streams(uslots),
            "useqs": us,
        })
    return in_maps


def assemble(results):
    """[P, NT, F] p-major slabs per core -> [N, F] full output."""
    parts = []
    for c in range(NC_COUNT):
        arr = np.asarray(results[c]["out"]).astype(np.float32)
        parts.append(arr.transpose(1, 0, 2).reshape(NSH_PAD, F)[:NSH])
    return np.concatenate(parts, axis=0)


def kernel(x, seq, useq, W1, W2):
    from concourse.bass_utils import run_bass_kernel_spmd

    in_maps = make_in_maps(x, seq, useq, W1, W2)
    nc = build_program()
    res = run_bass_kernel_spmd(nc, in_maps, core_ids=list(range(NC_COUNT)),
                               trace=False)
    return assemble(res.results)
